# revision 1
# baseline (speedup 1.0000x reference)
"""Corr1d-x-group cost-volume kernel for Trainium2 (8 NeuronCores, SPMD).

Data-parallel over batch N=8: core i processes batch i.

Per core (inputs [16, 256, 512] f32 each, output [108, 256, 512] f32):
  out[g*27+ch, h, w] = 0.25 * sum_c f1[g*4+c, h, w] * f2[g*4+c, h, w+ch-23]
with zero padding outside w in [0, 512).

Implementation:
  - Inputs are DMA-cast f32->f16 on load (SWDGE cast DMA).
  - SBUF layout per 8-row h-block: partitions = (channel(16), h8(8)) = 128.
  - 27 shifted products on VectorE (fp16 tensor_tensor, 2x perf mode;
    dual parity copies of the padded f2 tile keep slices 4B-aligned).
  - Channel reduction (sum over c of each group g) via TensorE matmul with a
    constant block-diagonal 0.25 weight matrix [128, 32]; 4 shifts packed
    into one PSUM bank via tile_position column groups -> fp32 PSUM.
  - ScalarE copies PSUM->SBUF, HWDGE DMA stores to DRAM.
"""

import os
import numpy as np

import concourse.bass as bass
import concourse.bacc as bacc
import concourse.mybir as mybir
import concourse.tile as tile
from concourse import bass_utils

N, C, H, W = 8, 16, 256, 512
G = 4
TOP_CH = 27
RADIUS = 13
PAD_SHIFT = -10  # shift s = ch - 23 for ch in [0, 27)
OUT_CH = G * TOP_CH  # 108
HB = 32  # h rows per block; 4 channels * 32 rows = 128 partitions
NBLK = H // HB
PADL = 24  # f2 padded tile: column = w + PADL (even so slices align)
F2W = PADL + W + 8  # 544 columns, covers w in [-24, 520)

_CACHED = {}


def _reduction_weights() -> np.ndarray:
    # lhsT [K=(c, h32)=128, M=h32=32]: sums the 4 channels of a group and
    # applies the 1/sumelems scale.
    w = np.zeros((128, 32), np.float16)
    for c in range(G):
        for hh in range(HB):
            w[c * HB + hh, hh] = 0.25
    return w


def _build_program() -> bass.Bass:
    # Bacc (not raw Bass): its compile() splits multi-sem sync waits, which
    # TRN2 hardware limits to one per instruction.
    nc = bacc.Bacc(
        "TRN2",
        target_bir_lowering=False,
        debug=False,
        enable_asserts=False,
        num_devices=N,
    )
    f16 = mybir.dt.float16
    f32 = mybir.dt.float32

    l_in = nc.dram_tensor("l_in", [C, H, W], f32, kind="ExternalInput")
    r_in = nc.dram_tensor("r_in", [C, H, W], f32, kind="ExternalInput")
    w_red = nc.dram_tensor("w_red", [128, 32], f16, kind="ExternalInput")
    out = nc.dram_tensor("out", [OUT_CH, H, W], f32, kind="ExternalOutput")

    # Output viewed as [ch(27), g(4), h*w]: one shift's store for an h-block
    # is [1, 4, HB*W] -> a 2-dim AP against the [128, 512] SBUF stage tile
    # whose partition-major order is (g, h32, w).
    out_v = out.ap().rearrange("(g c) h w -> c g (h w)", g=G)

    with tile.TileContext(nc) as tc:
        with (
            tc.tile_pool(name="wpool", bufs=1) as wpool,
            tc.tile_pool(name="inpool", bufs=2) as inpool,
            tc.tile_pool(name="prodpool", bufs=4) as prodpool,
            tc.tile_pool(name="obpool", bufs=3) as obpool,
            tc.tile_pool(name="psumpool", bufs=2, space="PSUM") as psumpool,
        ):
            wt = wpool.tile([128, 32], f16)
            nc.sync.dma_start(wt[:], w_red[:])

            for ib in range(NBLK):
                h0 = ib * HB
                f1s = []
                f2es = []
                f2os = []
                for g in range(G):
                    f1 = inpool.tile([128, W], f16, tag=f"f1_{g}")
                    nc.gpsimd.dma_start(
                        f1[:], l_in[g * G : (g + 1) * G, h0 : h0 + HB, :]
                    )
                    f1s.append(f1)

                    f2e = inpool.tile([128, F2W], f16, tag=f"f2e_{g}")
                    nc.vector.memset(f2e[:, 0:PADL], 0.0)
                    nc.vector.memset(f2e[:, PADL + W : F2W], 0.0)
                    nc.gpsimd.dma_start(
                        f2e[:, PADL : PADL + W],
                        r_in[g * G : (g + 1) * G, h0 : h0 + HB, :],
                    )
                    f2es.append(f2e)
                    # Odd-parity tile: same data at column = w + (PADL-1), so
                    # odd shifts read from a 4B-aligned start. Loaded with its
                    # own cast-DMA (a DVE shift-copy trips the sync-wait cap).
                    f2o = inpool.tile([128, F2W], f16, tag=f"f2o_{g}")
                    nc.vector.memset(f2o[:, 0 : PADL - 1], 0.0)
                    nc.vector.memset(f2o[:, PADL - 1 + W : F2W], 0.0)
                    nc.gpsimd.dma_start(
                        f2o[:, PADL - 1 : PADL - 1 + W],
                        r_in[g * G : (g + 1) * G, h0 : h0 + HB, :],
                    )
                    f2os.append(f2o)

                for ch in range(TOP_CH):
                    col = PADL + ch - (RADIUS - PAD_SHIFT)  # PADL + shift
                    psumt = psumpool.tile([128, W], f32, tag="psumt")
                    for g in range(G):
                        if col % 2 == 0:
                            src = f2es[g][:, col : col + W]
                        else:
                            src = f2os[g][:, col - 1 : col - 1 + W]
                        p = prodpool.tile([128, W], f16, tag="prod")
                        nc.vector.tensor_mul(p[:], f1s[g][:], src)
                        nc.tensor.matmul(
                            psumt[32 * g : 32 * (g + 1), :],
                            wt[:],
                            p[:],
                            start=True,
                            stop=True,
                            tile_position=(0, 32 * g),
                        )
                    ob = obpool.tile([128, W], f32, tag="ob")
                    nc.scalar.copy(ob[:], psumt[:])
                    nc.sync.dma_start(
                        out_v[ch : ch + 1, :, h0 * W : (h0 + HB) * W],
                        ob[:],
                    )
    nc.compile()
    return nc


def kernel(l_in: np.ndarray, r_in: np.ndarray) -> np.ndarray:
    assert l_in.shape == (N, C, H, W) and r_in.shape == (N, C, H, W)
    l_in = np.ascontiguousarray(l_in, dtype=np.float32)
    r_in = np.ascontiguousarray(r_in, dtype=np.float32)

    if "nc" not in _CACHED:
        _CACHED["nc"] = _build_program()
    nc = _CACHED["nc"]

    w_np = _reduction_weights()
    in_maps = [
        {
            "l_in": np.ascontiguousarray(l_in[i]),
            "r_in": np.ascontiguousarray(r_in[i]),
            "w_red": w_np,
        }
        for i in range(N)
    ]
    trace = bool(int(os.environ.get("CORR_KERNEL_TRACE", "0")))
    kwargs = {}
    tdir = os.environ.get("CORR_KERNEL_TRACE_DIR")
    if trace and tdir:
        os.makedirs(tdir, exist_ok=True)
        kwargs["tmpdir"] = tdir
    res = bass_utils.run_bass_kernel_spmd(
        nc, in_maps, core_ids=list(range(N)), trace=trace, **kwargs
    )
    _CACHED["last_result"] = res
    return np.stack([res.results[i]["out"] for i in range(N)], axis=0)



# revision 6
# speedup vs baseline: 1.1041x; 1.1041x over previous
"""Corr1d-x-group cost-volume kernel for Trainium2 (8 NeuronCores, SPMD).

Data-parallel over batch N=8: core i processes batch i.

Per core (inputs [16, 256, 512] f32 each, output [108, 256, 512] f32):
  out[g*27+ch, h, w] = 0.25 * sum_c f1[g*4+c, h, w] * f2[g*4+c, h, w+ch-23]
with zero padding outside w in [0, 512).

v3 design (vs. baseline):
  - h = hb*8 + hi. Per group g: partitions = (c(4), hb(32)), free = (hi(8), w).
    One fp16 tensor_tensor of FD=4096 per (group, shift) - 108 total - instead
    of 864 FD=512 ops.
  - No zero-padding of f2: shifted slices read junk from neighboring rows /
    guard columns; the wrapped product columns are memset to 0 before the
    channel-reduction matmul (true output there is exactly 0).
  - Odd-parity copy of f2 built on-chip (ScalarE) so odd shifts keep the
    DVE 2x perf mode; r_in is read from HBM only once (~25% less HBM read).
  - Channel reduction on TensorE: weight [128,32] = 0.25 * (c-sum, hb-identity),
    4 groups packed per PSUM bank via tile_position; psum tile [128,1024]
    holds (g,hb) x (hi-pair, w).
  - ScalarE copies PSUM->SBUF staging [128, 4096] f32 per shift; ONE 2 MB
    store per shift (27 total) whose DRAM AP merges to (g, 512KB-contiguous),
    16KB descriptors, alternating between the two HWDGE rings (sync+scalar).
  - A few shifts' multiplies run on GpSimd to offload the DVE.
"""

import os
import numpy as np

import concourse.bass as bass
import concourse.bacc as bacc
import concourse.mybir as mybir
import concourse.tile as tile
from concourse import bass_utils

N, C, H, W = 8, 16, 256, 512
G = 4
TOP_CH = 27
OUT_CH = G * TOP_CH  # 108
HB = 32   # h // 8 -> partition dim component
HI = 8    # h % 8  -> free dim component
FD = HI * W  # 4096
GUARD_L = 24
F2W = GUARD_L + FD + 8  # 4128
GPS_SIDX = (5, 12, 19, 26)  # shifts whose 4 multiplies run on GpSimd

_CACHED = {}


def _reduction_weights() -> np.ndarray:
    # lhsT [K=(c,hb)=128, M=hb=32]: sums the 4 channels of a group and
    # applies the 1/sumelems scale.
    w = np.zeros((128, 32), np.float16)
    for c in range(G):
        for hb in range(HB):
            w[c * HB + hb, hb] = 0.25
    return w


def _build_program() -> bass.Bass:
    # Bacc (not raw Bass): its compile() splits multi-sem sync waits, which
    # TRN2 hardware limits to one per instruction.
    nc = bacc.Bacc(
        "TRN2",
        target_bir_lowering=False,
        debug=False,
        enable_asserts=False,
        num_devices=N,
    )
    f16 = mybir.dt.float16
    f32 = mybir.dt.float32

    l_in = nc.dram_tensor("l_in", [C, H, W], f32, kind="ExternalInput")
    r_in = nc.dram_tensor("r_in", [C, H, W], f32, kind="ExternalInput")
    w_red = nc.dram_tensor("w_red", [128, 32], f16, kind="ExternalInput")
    out = nc.dram_tensor("out", [OUT_CH, H, W], f32, kind="ExternalOutput")

    # DRAM views. h = hb*8 + hi.
    l_v = l_in.ap().rearrange(
        "(g c) (hb hi) w -> g (c hb) (hi w)", g=G, hb=HB, hi=HI
    )
    r_v = r_in.ap().rearrange(
        "(g c) (hb hi) w -> g (c hb) (hi w)", g=G, hb=HB, hi=HI
    )
    out_v = out.ap().rearrange(
        "(g s) (hb hi) w -> s g hb (hi w)", g=G, s=TOP_CH, hb=HB, hi=HI
    )

    with tile.TileContext(nc) as tc:
        with (
            tc.tile_pool(name="wpool", bufs=1) as wpool,
            tc.tile_pool(name="inpool", bufs=1) as inpool,
            tc.tile_pool(name="prodpool", bufs=6) as prodpool,
            tc.tile_pool(name="stgpool", bufs=3) as stgpool,
            tc.tile_pool(name="psumpool", bufs=4, space="PSUM") as psumpool,
        ):
            wt = wpool.tile([128, 32], f16)
            nc.sync.dma_start(wt[:], w_red[:])

            f1s, f2es, f2os = [], [], []
            for g in range(G):
                f1 = inpool.tile([128, FD], f16, tag=f"f1_{g}")
                nc.gpsimd.dma_start(f1[:], l_v[g : g + 1])
                f1s.append(f1)
                f2e = inpool.tile([128, F2W], f16, tag=f"f2e_{g}")
                nc.gpsimd.dma_start(
                    f2e[:, GUARD_L : GUARD_L + FD], r_v[g : g + 1]
                )
                f2es.append(f2e)
                # Odd-parity copy: f2o[:, col] = f2e[:, col+1], so odd shifts
                # read 4B-aligned starts (keeps DVE 2x perf mode).
                f2o = inpool.tile([128, F2W], f16, tag=f"f2o_{g}")
                nc.scalar.copy(
                    f2o[:, GUARD_L - 1 : GUARD_L + 2 + FD],
                    f2e[:, GUARD_L : GUARD_L + 3 + FD],
                )
                f2os.append(f2o)

            for s_idx in range(TOP_CH):
                s = s_idx - 23
                stg = stgpool.tile([128, FD], f32, tag="stg")
                ps = []
                for g in range(G):
                    if s % 2 == 0:
                        src = f2es[g][:, GUARD_L + s : GUARD_L + s + FD]
                    else:
                        src = f2os[g][:, GUARD_L - 1 + s : GUARD_L - 1 + s + FD]
                    p = prodpool.tile([128, FD], f16, tag="prod")
                    eng = nc.gpsimd if s_idx in GPS_SIDX else nc.vector
                    eng.tensor_mul(p[:], f1s[g][:], src)
                    # Zero the wrapped columns (true output is 0 there).
                    p3 = p[:].rearrange("a (hi w) -> a hi w", hi=HI)
                    if s < 0:
                        nc.vector.memset(p3[:, :, 0 : -s], 0.0)
                    elif s > 0:
                        nc.vector.memset(p3[:, :, W - s : W], 0.0)
                    ps.append(p)
                for j in range(4):  # hi pairs (2j, 2j+1)
                    pt = psumpool.tile([128, 1024], f32, tag="pt")
                    for k in range(2):
                        hi = 2 * j + k
                        for g in range(G):
                            nc.tensor.matmul(
                                pt[32 * g : 32 * g + 32, 512 * k : 512 * (k + 1)],
                                wt[:],
                                ps[g][:, 512 * hi : 512 * (hi + 1)],
                                start=True,
                                stop=True,
                                tile_position=(0, 32 * g),
                            )
                    nc.scalar.copy(stg[:, 1024 * j : 1024 * (j + 1)], pt[:])
                dma_eng = nc.sync if s_idx % 2 == 0 else nc.scalar
                dma_eng.dma_start(out_v[s_idx : s_idx + 1], stg[:])
    nc.compile()
    return nc


def kernel(l_in: np.ndarray, r_in: np.ndarray) -> np.ndarray:
    assert l_in.shape == (N, C, H, W) and r_in.shape == (N, C, H, W)
    l_in = np.ascontiguousarray(l_in, dtype=np.float32)
    r_in = np.ascontiguousarray(r_in, dtype=np.float32)

    if "nc" not in _CACHED:
        _CACHED["nc"] = _build_program()
    nc = _CACHED["nc"]

    w_np = _reduction_weights()
    in_maps = [
        {
            "l_in": np.ascontiguousarray(l_in[i]),
            "r_in": np.ascontiguousarray(r_in[i]),
            "w_red": w_np,
        }
        for i in range(N)
    ]
    trace = bool(int(os.environ.get("CORR_KERNEL_TRACE", "0")))
    kwargs = {}
    tdir = os.environ.get("CORR_KERNEL_TRACE_DIR")
    if trace and tdir:
        os.makedirs(tdir, exist_ok=True)
        kwargs["tmpdir"] = tdir
    res = bass_utils.run_bass_kernel_spmd(
        nc, in_maps, core_ids=list(range(N)), trace=trace, **kwargs
    )
    _CACHED["last_result"] = res
    return np.stack([res.results[i]["out"] for i in range(N)], axis=0)
